# revision 1
# baseline (speedup 1.0000x reference)
"""GAT (2-layer, PyG-style) on 8 Trainium2 NeuronCores via Bass/Tile.

Strategy (dst-sharded message passing):
  - Destination nodes are partitioned into 8 contiguous chunks (6250/core);
    each core owns all edges incident to its dst chunk, grouped into 128-dst
    windows, each window's edge list split by src half (int16 gather range)
    and padded to 128-edge chunks (uniform across cores -> one SPMD program).
  - Phase 1 (replicated on every core): h_ext = x @ [W1 | W1@Asrc | W1@Adst]
    -> gather table h_tab [npad, 320] holding [h(256) | aS(8) | aD(8) | pad].
  - aD_local [6272, 64]: own-node rows of the aD columns, copied out of
    h_tab with a partition_id-dependent dynamic DMA offset.
  - Phase 2 (per window): gpsimd.dma_gather of h+aS rows by src (two calls:
    low/high table half) and aD rows by local dst; segment softmax +
    weighted aggregation via one-hot S matrices on the PE (denominator as a
    second matmul stream into the same PSUM tile); relu(out+b1) -> h1;
    immediately h2_ext = h1 @ [W2 | W2@a2src | W2@a2dst]; h2 rows -> DRAM,
    aS2/aD2 -> a2_local.
  - AllGather of h2 (the only collective).
  - Phase 3: same windowed machinery for layer 2 (single head); aS2 comes
    from a tensor_tensor_reduce against the gathered h2 rows; log_softmax
    (deferred ln) rows DMA'd to the output.
"""
import sys

for _p in ("/opt/trn_rl_repo", "/opt/pypackages"):
    if _p not in sys.path:
        sys.path.insert(0, _p)

import numpy as np
from concourse import bacc, bass, mybir, tile
from concourse.masks import make_identity

P = 128
F32 = mybir.dt.float32
I16 = mybir.dt.int16
HALF = 32768

# ---- problem constants (nn_GAT_60000693125135) ----
N = 50000
IN_DIM = 256
H1 = 8          # heads layer 1
HID = 32        # per-head dim layer 1
HC1 = H1 * HID  # 256
OUT = 64
NCORES = 8
NEG_SLOPE = 0.2


def _cdiv(a, b):
    return -(-a // b)


def _wrap16(vals, nidx):
    """int16 idx list -> [128, nidx//16] wrap-16 layout, replicated x8."""
    a = np.asarray(vals, np.int16).reshape(nidx // 16, 16).T  # [16, cols]
    return np.tile(a, (8, 1))


# ----------------------------------------------------------------------------
# Host-side preprocessing.
# ----------------------------------------------------------------------------
def prep_edges(edge_index, n, ncores):
    """Shard + window + src-half-split the edge list.

    Returns dict with per-core arrays:
      srclo16 [ncores, 128, 8*CTlo]  (int16, wrap-16 per window)
      srchi16 [ncores, 128, 8*CThi]
      dloc16  [ncores, 128, 8*CT]
      d128    [ncores, 128, CT]     (f32; 999 sentinel on pads)
    and CWlo, CWhi lists (len nw).
    """
    e0 = edge_index[0].astype(np.int64)
    e1 = edge_index[1].astype(np.int64)
    loops = np.arange(n, dtype=np.int64)
    src = np.concatenate([e0, loops])
    dst = np.concatenate([e1, loops])

    nchunk = n // ncores
    nw = _cdiv(nchunk, P)
    core = dst // nchunk
    dloc = dst - core * nchunk
    w = dloc // P
    hi = (src >= HALF).astype(np.int64)
    # group id: (core, window, half)
    gid = (core * nw + w) * 2 + hi
    ngroups = ncores * nw * 2
    cnt = np.bincount(gid, minlength=ngroups).reshape(ncores, nw, 2)
    CWlo = _cdiv(cnt[:, :, 0].max(axis=0), P)          # may be 0
    CWhi = _cdiv(cnt[:, :, 1].max(axis=0), P)
    CW = CWlo + CWhi
    assert CW.min() >= 1
    CTlo, CThi, CT = int(CWlo.sum()), int(CWhi.sum()), int(CW.sum())

    order = np.argsort(gid, kind="stable")
    gid_s = gid[order]
    starts = np.concatenate([[0], np.cumsum(np.bincount(gid_s, minlength=ngroups))])
    pos = np.arange(order.size) - starts[gid_s]

    # slot (within window): lo edges at [0, CWlo*128), hi at [CWlo*128, CW*128)
    c_s = core[order]
    w_s = w[order]
    hi_s = hi[order]
    slot = pos + hi_s * (CWlo[w_s] * P)

    # per-(core,window) slot-value arrays
    srcv = np.zeros((ncores, nw, np.max(CW) * P), np.int64)   # src - half*HALF
    dlocv = np.zeros((ncores, nw, np.max(CW) * P), np.int64)
    d128v = np.full((ncores, nw, np.max(CW) * P), 999.0, np.float32)
    srcv[c_s, w_s, slot] = src[order] - hi_s * HALF
    dlocv[c_s, w_s, slot] = dloc[order]
    d128v[c_s, w_s, slot] = (dloc[order] % P).astype(np.float32)

    srclo16 = np.zeros((ncores, P, 8 * CTlo), np.int16)
    srchi16 = np.zeros((ncores, P, 8 * CThi), np.int16)
    dloc16 = np.zeros((ncores, P, 8 * CT), np.int16)
    d128A = np.full((ncores, P, CT), 999.0, np.float32)
    olo = ohi = oall = 0
    for wi in range(nw):
        nlo, nhi, nall = int(CWlo[wi]) * P, int(CWhi[wi]) * P, int(CW[wi]) * P
        for c in range(ncores):
            if nlo:
                srclo16[c, :, 8 * olo:8 * (olo + nlo // P)] = _wrap16(srcv[c, wi, :nlo], nlo)
            if nhi:
                srchi16[c, :, 8 * ohi:8 * (ohi + nhi // P)] = _wrap16(srcv[c, wi, nlo:nall], nhi)
            dloc16[c, :, 8 * oall:8 * (oall + nall // P)] = _wrap16(dlocv[c, wi, :nall], nall)
            d128A[c, :, oall:oall + nall // P] = d128v[c, wi, :nall].reshape(nall // P, P).T
    # fmt: off
        olo += nlo // P; ohi += nhi // P; oall += nall // P
    # fmt: on
    return dict(srclo16=srclo16, srchi16=srchi16, dloc16=dloc16, d128=d128A,
                CWlo=[int(v) for v in CWlo], CWhi=[int(v) for v in CWhi])


# ----------------------------------------------------------------------------
# Kernel builder (SPMD program, same for all cores).
# ----------------------------------------------------------------------------
def build_nc(cfg):
    n = cfg["N"]; in_dim = cfg["IN"]; hc1 = cfg["HC1"]; h1 = cfg["H1"]
    hid = cfg["HID"]; out_dim = cfg["OUT"]; ncores = cfg["NCORES"]
    neg = cfg["NEG"]
    CWlo, CWhi = cfg["CWlo"], cfg["CWhi"]
    CW = [a + b for a, b in zip(CWlo, CWhi)]

    h2c = 2 * h1
    TROW = hc1 + h2c + (-(hc1 + h2c) % 64)   # gather row f32 count (%64 -> 256B)
    assert out_dim == 64, "h2 gather rows must be 256B"
    nchunk = n // ncores
    nw = _cdiv(nchunk, P)
    assert len(CW) == nw
    CTlo, CThi, CT = sum(CWlo), sum(CWhi), sum(CW)
    ntiles = _cdiv(n, P)
    npad = ntiles * P
    nlpad = _cdiv(nchunk, P) * P             # aD_local rows
    kt1 = _cdiv(in_dim, P)
    ckt = _cdiv(hc1, P)
    cmax = max(CW)
    NB = 8

    nc = bacc.Bacc(None, target_bir_lowering=False, debug=False,
                   num_devices=ncores)

    # ---- I/O ----
    xT_in = nc.dram_tensor("xT", [in_dim, npad], F32, kind="ExternalInput")
    w1_in = nc.dram_tensor("W1", [in_dim, hc1], F32, kind="ExternalInput")
    w1T_in = nc.dram_tensor("W1T", [hc1, in_dim], F32, kind="ExternalInput")
    amat_in = nc.dram_tensor("Amat", [hc1, h2c], F32, kind="ExternalInput")
    w2_in = nc.dram_tensor("W2", [hc1, out_dim], F32, kind="ExternalInput")
    w2T_in = nc.dram_tensor("W2T", [out_dim, hc1], F32, kind="ExternalInput")
    a2_in = nc.dram_tensor("A2", [out_dim, 2], F32, kind="ExternalInput")
    a2s_in = nc.dram_tensor("a2srep", [P, out_dim], F32, kind="ExternalInput")
    b1r_in = nc.dram_tensor("b1r", [P, hc1], F32, kind="ExternalInput")
    b2r_in = nc.dram_tensor("b2r", [P, out_dim], F32, kind="ExternalInput")
    iod_in = nc.dram_tensor("iod", [P, P + CT], F32, kind="ExternalInput")
    slo_in = nc.dram_tensor("srclo16", [P, 8 * CTlo], I16, kind="ExternalInput")
    shi_in = nc.dram_tensor("srchi16", [P, max(8 * CThi, 16)], I16, kind="ExternalInput")
    dlo_in = nc.dram_tensor("dloc16", [P, 8 * CT], I16, kind="ExternalInput")
    out_ext = nc.dram_tensor("out", [nchunk, out_dim], F32, kind="ExternalOutput")

    with tile.TileContext(nc) as tc:
        with (
            tc.tile_pool(name="dram", bufs=1, space="DRAM") as dram,
            tc.tile_pool(name="const", bufs=1) as cpool,
            tc.tile_pool(name="xst", bufs=2) as xpool,
            tc.tile_pool(name="hst", bufs=2) as hpool,
            tc.tile_pool(name="gbuf", bufs=2) as gpool,
            tc.tile_pool(name="sbuf2", bufs=2) as spool,
            tc.tile_pool(name="small", bufs=3) as smpool,
            tc.tile_pool(name="adl", bufs=8) as adpool,
            tc.tile_pool(name="w0", bufs=1) as w0pool,
            tc.tile_pool(name="psA", bufs=2, space="PSUM") as psA,
            tc.tile_pool(name="psB", bufs=2, space="PSUM") as psB,
            tc.tile_pool(name="psC", bufs=2, space="PSUM") as psC,
        ):
            # ---- DRAM scratch ----
            h_tab = dram.tile([npad, TROW], F32)
            aD_local = dram.tile([nlpad, 64], F32)
            a2_local = dram.tile([nlpad, 64], F32)
            h2_mine = dram.tile([nchunk, out_dim], F32)
            cc_space = "Shared" if ncores > 4 else "Local"
            h2_tab = dram.tile([n, out_dim], F32, addr_space=cc_space)

            # ---- resident constants ----
            iod_t = cpool.tile([P, P + CT], F32)
            nc.sync.dma_start(out=iod_t[:], in_=iod_in[:])
            iota_t = iod_t[:, 0:P]
            d128t = iod_t[:, P:]
            ident = cpool.tile([P, P], F32)
            make_identity(nc, ident[:])
            b1r = cpool.tile([P, hc1], F32)
            nc.sync.dma_start(out=b1r[:], in_=b1r_in[:])
            b2r = cpool.tile([P, out_dim], F32)
            nc.sync.dma_start(out=b2r[:], in_=b2r_in[:])
            a2srep = cpool.tile([P, out_dim], F32)
            nc.sync.dma_start(out=a2srep[:], in_=a2s_in[:])
            slo = cpool.tile([P, 8 * CTlo], I16)
            nc.sync.dma_start(out=slo[:], in_=slo_in[:])
            shi = cpool.tile([P, max(8 * CThi, 16)], I16)
            nc.sync.dma_start(out=shi[:], in_=shi_in[:])
            dlo = cpool.tile([P, 8 * CT], I16)
            nc.sync.dma_start(out=dlo[:], in_=dlo_in[:])

            # ---- phase 0: extended weights ----
            w1ext = cpool.tile([P, kt1, hc1 + h2c], F32)
            for kt in range(kt1):
                kp = min(P, in_dim - kt * P)
                nc.sync.dma_start(out=w1ext[:kp, kt, 0:hc1], in_=w1_in[kt * P:kt * P + kp, :])
            w1T_sb = w0pool.tile([P, ckt, in_dim], F32)
            amat_sb = w0pool.tile([P, ckt, h2c], F32)
            for c in range(ckt):
                cp = min(P, hc1 - c * P)
                nc.sync.dma_start(out=w1T_sb[:cp, c, :], in_=w1T_in[c * P:c * P + cp, :])
                nc.sync.dma_start(out=amat_sb[:cp, c, :], in_=amat_in[c * P:c * P + cp, :])
            for kt in range(kt1):
                kp = min(P, in_dim - kt * P)
                wps = psB.tile([P, h2c], F32, tag="tp")
                for c in range(ckt):
                    cp = min(P, hc1 - c * P)
                    nc.tensor.matmul(out=wps[:kp, :], lhsT=w1T_sb[:cp, c, kt * P:kt * P + kp],
                                     rhs=amat_sb[:cp, c, :], start=(c == 0), stop=(c == ckt - 1))
                nc.scalar.copy(out=w1ext[:kp, kt, hc1:], in_=wps[:kp, :])

            w2ext = cpool.tile([P, ckt, out_dim + 2], F32)
            w2T_sb = w0pool.tile([out_dim, hc1], F32)
            a2_sb = w0pool.tile([out_dim, 2], F32)
            nc.sync.dma_start(out=w2T_sb[:], in_=w2T_in[:])
            nc.sync.dma_start(out=a2_sb[:], in_=a2_in[:])
            for c in range(ckt):
                cp = min(P, hc1 - c * P)
                nc.sync.dma_start(out=w2ext[:cp, c, 0:out_dim], in_=w2_in[c * P:c * P + cp, :])
                wps2 = psB.tile([P, 2], F32, tag="tp")
                nc.tensor.matmul(out=wps2[:cp, :], lhsT=w2T_sb[:, c * P:c * P + cp],
                                 rhs=a2_sb[:], start=True, stop=True)
                nc.scalar.copy(out=w2ext[:cp, c, out_dim:], in_=wps2[:cp, :])

            # ---- phase 1: h_ext = x @ w1ext -> h_tab [npad, TROW] ----
            for g in range(_cdiv(ntiles, NB)):
                nt0 = g * NB
                nb = min(NB, ntiles - nt0)
                xst = xpool.tile([P, kt1, NB * P], F32, tag="xst")
                for kt in range(kt1):
                    kp = min(P, in_dim - kt * P)
                    nc.sync.dma_start(out=xst[:kp, kt, 0:nb * P],
                                      in_=xT_in[kt * P:kt * P + kp, nt0 * P:nt0 * P + nb * P])
                hstg = hpool.tile([P, NB, hc1 + h2c], F32, tag="hst")
                for j in range(nb):
                    ps = psA.tile([P, hc1 + h2c], F32, tag="mm")
                    for kt in range(kt1):
                        kp = min(P, in_dim - kt * P)
                        nc.tensor.matmul(out=ps[:], lhsT=xst[:kp, kt, j * P:(j + 1) * P],
                                         rhs=w1ext[:kp, kt, :], start=(kt == 0), stop=(kt == kt1 - 1))
                    nc.scalar.copy(out=hstg[:, j, :], in_=ps[:])
                hv = h_tab[nt0 * P:(nt0 + nb) * P, 0:hc1 + h2c].rearrange(
                    "(j p) c -> p j c", p=P)
                nc.sync.dma_start(out=hv, in_=hstg[:, 0:nb, :])

            # ---- aD_local: own-node aD rows via dynamic-offset DMA ----
            pid_rows = nc.sync.snap(nc.sync.partition_id() * nchunk)
            for w in range(nw):
                rows = min(P, nchunk - w * P)
                bnc = adpool.tile([P, h1], F32, tag="adl")
                nc.sync.dma_start(
                    out=bnc[:rows, :],
                    in_=h_tab[bass.ds(pid_rows + w * P, rows), hc1 + h1:hc1 + h2c])
                nc.sync.dma_start(out=aD_local[w * P:w * P + rows, 0:h1], in_=bnc[:rows, :])

            stop = cfg.get("STOP", "")

            def bounce_out(src_dram):
                for w in range(nw):
                    rows = min(P, nchunk - w * P)
                    dbg = smpool.tile([P, out_dim], F32, tag="z")
                    nc.sync.dma_start(out=dbg[:rows, :],
                                      in_=src_dram[w * P:w * P + rows, 0:out_dim])
                    nc.sync.dma_start(out=out_ext[w * P:w * P + rows, :],
                                      in_=dbg[:rows, :])

            if stop == "phase1":
                bounce_out(h_tab)
                return nc

            # ---- phase 2: layer-1 edge aggregation per dst window ----
            olo = oall = 0
            for w in range(nw):
                Clo, Chi, C = CWlo[w], CWhi[w], CW[w]
                rows = min(P, nchunk - w * P)
                G = gpool.tile([P, cmax, TROW], F32, tag="G")
                if Clo:
                    nc.gpsimd.dma_gather(
                        out_ap=G[:, 0:Clo, :], in_ap=h_tab[:],
                        idxs_ap=slo[:, 8 * olo:8 * (olo + Clo)],
                        num_idxs=Clo * P, num_idxs_reg=Clo * P, elem_size=TROW,
                        single_packet=False)
                if Chi:
                    nc.gpsimd.dma_gather(
                        out_ap=G[:, Clo:C, :], in_ap=h_tab[HALF:, :],
                        idxs_ap=shi[:, 8 * (oall - olo):8 * (oall - olo + Chi)],
                        num_idxs=Chi * P, num_idxs_reg=Chi * P, elem_size=TROW,
                        single_packet=False)
                aDb = spool.tile([P, cmax, 64], F32, tag="aDb")
                nc.gpsimd.dma_gather(
                    out_ap=aDb[:, 0:C, :], in_ap=aD_local[:],
                    idxs_ap=dlo[:, 8 * oall:8 * (oall + C)],
                    num_idxs=C * P, num_idxs_reg=C * P, elem_size=64,
                    single_packet=False)
                if stop == "gather":
                    nc.sync.dma_start(out=out_ext[w * P:w * P + rows, :],
                                      in_=G[:rows, 0, 0:out_dim])
                    olo += Clo; oall += C
                    continue
                S = spool.tile([P, cmax, P], F32, tag="S")
                nc.vector.tensor_tensor(
                    out=S[:, 0:C, :],
                    in0=d128t[:, oall:oall + C].unsqueeze(-1).to_broadcast((P, C, P)),
                    in1=iota_t.unsqueeze(1).to_broadcast((P, C, P)),
                    op=mybir.AluOpType.is_equal)
                # p = exp(lrelu(aS + aD)), written back over the aS columns
                # of G so one matmul covers features + denominator.
                nc.vector.tensor_add(out=G[:, 0:C, hc1:hc1 + h1],
                                     in0=G[:, 0:C, hc1:hc1 + h1],
                                     in1=aDb[:, 0:C, 0:h1])
                nc.vector.scalar_tensor_tensor(
                    out=G[:, 0:C, hc1:hc1 + h1], in0=G[:, 0:C, hc1:hc1 + h1],
                    scalar=neg, in1=G[:, 0:C, hc1:hc1 + h1],
                    op0=mybir.AluOpType.mult, op1=mybir.AluOpType.max)
                nc.scalar.activation(out=G[:, 0:C, hc1:hc1 + h1],
                                     in_=G[:, 0:C, hc1:hc1 + h1],
                                     func=mybir.ActivationFunctionType.Exp)
                for h in range(h1):
                    nc.vector.tensor_tensor(
                        out=G[:, 0:C, h * hid:(h + 1) * hid],
                        in0=G[:, 0:C, h * hid:(h + 1) * hid],
                        in1=G[:, 0:C, hc1 + h:hc1 + h + 1].to_broadcast((P, C, hid)),
                        op=mybir.AluOpType.mult)
                ops = psA.tile([P, hc1 + h1], F32, tag="mm")
                for k in range(C):
                    nc.tensor.matmul(out=ops[:], lhsT=S[:, k, :], rhs=G[:, k, 0:hc1 + h1],
                                     start=(k == 0), stop=(k == C - 1))
                rec = smpool.tile([P, h1], F32, tag="rec")
                nc.vector.reciprocal(out=rec[:], in_=ops[:, hc1:hc1 + h1])
                h1w = spool.tile([P, hc1], F32, tag="h1w")
                nc.vector.tensor_tensor(
                    out=h1w[:].rearrange("p (h j) -> p h j", h=h1),
                    in0=ops[:, 0:hc1].rearrange("p (h j) -> p h j", h=h1),
                    in1=rec[:].unsqueeze(-1).to_broadcast((P, h1, hid)),
                    op=mybir.AluOpType.mult)
                nc.vector.tensor_add(out=h1w[:], in0=h1w[:], in1=b1r[:])
                nc.vector.tensor_scalar(out=h1w[:], in0=h1w[:], scalar1=0.0, scalar2=None,
                                        op0=mybir.AluOpType.max)
                if stop == "smm":
                    nc.sync.dma_start(out=out_ext[w * P:w * P + rows, :],
                                      in_=h1w[:rows, 0:out_dim])
                    olo += Clo; oall += C
                    continue
                # layer-2 row prep: h2_ext = h1 @ w2ext
                h1T = spool.tile([P, ckt, P], F32, tag="h1T")
                for c in range(ckt):
                    cp = min(P, hc1 - c * P)
                    tp = psB.tile([P, P], F32, tag="tp")
                    nc.tensor.transpose(tp[:cp, :], h1w[:, c * P:c * P + cp], ident[:])
                    nc.scalar.copy(out=h1T[:cp, c, :], in_=tp[:cp, :])
                h2ps = psC.tile([P, out_dim + 2], F32, tag="h2")
                for c in range(ckt):
                    cp = min(P, hc1 - c * P)
                    nc.tensor.matmul(out=h2ps[:], lhsT=h1T[:cp, c, :], rhs=w2ext[:cp, c, :],
                                     start=(c == 0), stop=(c == ckt - 1))
                h2sb = smpool.tile([P, out_dim + 2], F32, tag="h2sb")
                nc.scalar.copy(out=h2sb[:], in_=h2ps[:])
                nc.sync.dma_start(out=h2_mine[w * P:w * P + rows, :], in_=h2sb[:rows, 0:out_dim])
                nc.sync.dma_start(out=a2_local[w * P:w * P + rows, 0:2],
                                  in_=h2sb[:rows, out_dim:])
                olo += Clo; oall += C

            if stop in ("gather", "smm"):
                return nc
            if stop == "phase2":
                bounce_out(h2_mine)
                return nc

            # ---- all-gather h2 ----
            nc.gpsimd.collective_compute(
                "AllGather", mybir.AluOpType.bypass,
                replica_groups=[list(range(ncores))],
                ins=[h2_mine[:].opt()], outs=[h2_tab[:].opt()])

            if stop == "cc":
                bounce_out(h2_tab)
                return nc

            # ---- phase 3: layer-2 edge aggregation + log_softmax ----
            t_all = cpool.tile([P, nw, out_dim], F32)
            s_all = cpool.tile([P, nw], F32)
            olo = oall = 0
            for w in range(nw):
                Clo, Chi, C = CWlo[w], CWhi[w], CW[w]
                rows = min(P, nchunk - w * P)
                G2 = gpool.tile([P, cmax, out_dim], F32, tag="G")
                if Clo:
                    nc.gpsimd.dma_gather(
                        out_ap=G2[:, 0:Clo, :], in_ap=h2_tab[:],
                        idxs_ap=slo[:, 8 * olo:8 * (olo + Clo)],
                        num_idxs=Clo * P, num_idxs_reg=Clo * P, elem_size=out_dim,
                        single_packet=False)
                if Chi:
                    nc.gpsimd.dma_gather(
                        out_ap=G2[:, Clo:C, :], in_ap=h2_tab[HALF:, :],
                        idxs_ap=shi[:, 8 * (oall - olo):8 * (oall - olo + Chi)],
                        num_idxs=Chi * P, num_idxs_reg=Chi * P, elem_size=out_dim,
                        single_packet=False)
                aDb2 = spool.tile([P, cmax, 64], F32, tag="aDb")
                nc.gpsimd.dma_gather(
                    out_ap=aDb2[:, 0:C, :], in_ap=a2_local[:],
                    idxs_ap=dlo[:, 8 * oall:8 * (oall + C)],
                    num_idxs=C * P, num_idxs_reg=C * P, elem_size=64,
                    single_packet=False)
                S = spool.tile([P, cmax, P], F32, tag="S")
                nc.vector.tensor_tensor(
                    out=S[:, 0:C, :],
                    in0=d128t[:, oall:oall + C].unsqueeze(-1).to_broadcast((P, C, P)),
                    in1=iota_t.unsqueeze(1).to_broadcast((P, C, P)),
                    op=mybir.AluOpType.is_equal)
                # aS2_e = sum_c G2[e,:,c]*a2src[c]
                tmp2 = spool.tile([P, cmax, out_dim], F32, tag="tmp2")
                nc.vector.tensor_tensor(
                    out=tmp2[:, 0:C, :], in0=G2[:, 0:C, :],
                    in1=a2srep[:].unsqueeze(1).to_broadcast((P, C, out_dim)),
                    op=mybir.AluOpType.mult)
                p2 = smpool.tile([P, cmax], F32, tag="pe")
                nc.vector.tensor_reduce(out=p2[:, 0:C], in_=tmp2[:, 0:C, :],
                                        axis=mybir.AxisListType.X,
                                        op=mybir.AluOpType.add)
                nc.vector.tensor_add(out=p2[:, 0:C], in0=p2[:, 0:C],
                                     in1=aDb2[:, 0:C, 1].squeeze())
                nc.vector.scalar_tensor_tensor(
                    out=p2[:, 0:C], in0=p2[:, 0:C], scalar=neg, in1=p2[:, 0:C],
                    op0=mybir.AluOpType.mult, op1=mybir.AluOpType.max)
                nc.scalar.activation(out=p2[:, 0:C], in_=p2[:, 0:C],
                                     func=mybir.ActivationFunctionType.Exp)
                nc.vector.tensor_tensor(
                    out=G2[:, 0:C, :], in0=G2[:, 0:C, :],
                    in1=p2[:, 0:C].unsqueeze(-1).to_broadcast((P, C, out_dim)),
                    op=mybir.AluOpType.mult)
                ops2 = psA.tile([P, out_dim], F32, tag="mm")
                den2 = psA.tile([P, 1], F32, tag="den")
                for k in range(C):
                    nc.tensor.matmul(out=ops2[:], lhsT=S[:, k, :], rhs=G2[:, k, :],
                                     start=(k == 0), stop=(k == C - 1))
                    nc.tensor.matmul(out=den2[:], lhsT=S[:, k, :],
                                     rhs=p2[:, k:k + 1],
                                     start=(k == 0), stop=(k == C - 1))
                rec2 = smpool.tile([P, 1], F32, tag="rec")
                nc.vector.reciprocal(out=rec2[:], in_=den2[:])
                z = smpool.tile([P, out_dim], F32, tag="z")
                nc.vector.tensor_tensor(out=z[:], in0=ops2[:],
                                        in1=rec2[:].to_broadcast((P, out_dim)),
                                        op=mybir.AluOpType.mult)
                nc.vector.tensor_add(out=z[:], in0=z[:], in1=b2r[:])
                negmax = smpool.tile([P, 1], F32, tag="rec")
                nc.vector.tensor_reduce(out=negmax[:], in_=z[:], axis=mybir.AxisListType.X,
                                        op=mybir.AluOpType.max, negate=True)
                nc.vector.tensor_scalar(out=t_all[:, w, :], in0=z[:], scalar1=negmax[:],
                                        scalar2=None, op0=mybir.AluOpType.add)
                esc = smpool.tile([P, out_dim], F32, tag="z")
                nc.scalar.activation(out=esc[:], in_=t_all[:, w, :],
                                     func=mybir.ActivationFunctionType.Exp,
                                     accum_out=s_all[:, w:w + 1])
                olo += Clo; oall += C
            # epilogue: res = t - ln(s)
            lns = cpool.tile([P, nw], F32)
            nc.scalar.activation(out=lns[:], in_=s_all[:],
                                 func=mybir.ActivationFunctionType.Ln)
            for w in range(nw):
                rows = min(P, nchunk - w * P)
                res = smpool.tile([P, out_dim], F32, tag="z")
                nc.vector.tensor_scalar(out=res[:], in0=t_all[:, w, :], scalar1=lns[:, w:w + 1],
                                        scalar2=None, op0=mybir.AluOpType.subtract)
                nc.sync.dma_start(out=out_ext[w * P:w * P + rows, :], in_=res[:rows, :])

    return nc


# ----------------------------------------------------------------------------
# Host-side input packing.
# ----------------------------------------------------------------------------
def make_in_maps(inputs, cfg):
    n = cfg["N"]; in_dim = cfg["IN"]; hc1 = cfg["HC1"]; h1 = cfg["H1"]
    hid = cfg["HID"]; ncores = cfg["NCORES"]

    x = np.asarray(inputs["x"], np.float32)
    ei = np.asarray(inputs["edge_index"])
    W1 = np.asarray(inputs["W1"], np.float32)
    a_src1 = np.asarray(inputs["a_src1"], np.float32)
    a_dst1 = np.asarray(inputs["a_dst1"], np.float32)
    b1 = np.asarray(inputs["b1"], np.float32)
    W2 = np.asarray(inputs["W2"], np.float32)
    a_src2 = np.asarray(inputs["a_src2"], np.float32)
    a_dst2 = np.asarray(inputs["a_dst2"], np.float32)
    b2 = np.asarray(inputs["b2"], np.float32)

    ntiles = _cdiv(n, P)
    npad = ntiles * P
    xT = np.zeros((in_dim, npad), np.float32)
    xT[:, :n] = x.T

    amat = np.zeros((hc1, 2 * h1), np.float32)
    for h in range(h1):
        amat[h * hid:(h + 1) * hid, h] = a_src1[h]
        amat[h * hid:(h + 1) * hid, h1 + h] = a_dst1[h]

    a2 = np.stack([a_src2[0], a_dst2[0]], axis=1).astype(np.float32)

    pe = prep_edges(ei, n, ncores)
    cfg["CWlo"], cfg["CWhi"] = pe["CWlo"], pe["CWhi"]
    CT = sum(cfg["CWlo"]) + sum(cfg["CWhi"])

    common = {
        "W1": W1, "W1T": np.ascontiguousarray(W1.T),
        "Amat": amat, "W2": W2, "W2T": np.ascontiguousarray(W2.T), "A2": a2,
        "a2srep": np.tile(a_src2[0][None, :], (P, 1)).astype(np.float32),
        "b1r": np.tile(b1[None, :], (P, 1)).astype(np.float32),
        "b2r": np.tile(b2[None, :], (P, 1)).astype(np.float32),
        "xT": xT,
    }
    iota = np.tile(np.arange(P, dtype=np.float32)[None, :], (P, 1))
    in_maps = []
    for c in range(ncores):
        m = dict(common)
        m["srclo16"] = np.ascontiguousarray(pe["srclo16"][c])
        shi = pe["srchi16"][c]
        if shi.shape[1] == 0:
            shi = np.zeros((P, 16), np.int16)
        m["srchi16"] = np.ascontiguousarray(shi)
        m["dloc16"] = np.ascontiguousarray(pe["dloc16"][c])
        m["iod"] = np.ascontiguousarray(
            np.concatenate([iota, pe["d128"][c]], axis=1))
        in_maps.append(m)
    return in_maps


DEFAULT_CFG = dict(N=N, IN=IN_DIM, HC1=HC1, H1=H1, HID=HID, OUT=OUT,
                   NCORES=NCORES, NEG=NEG_SLOPE)

TRACE = False
LAST_RESULTS = None


def kernel(**inputs) -> np.ndarray:
    global LAST_RESULTS
    from concourse.bass_utils import run_bass_kernel_spmd

    cfg = dict(DEFAULT_CFG)
    in_maps = make_in_maps(inputs, cfg)
    nc = build_nc(cfg)
    if not nc.is_finalized():
        nc.finalize()
    res = run_bass_kernel_spmd(nc, in_maps, core_ids=list(range(cfg["NCORES"])),
                               trace=TRACE)
    LAST_RESULTS = res
    outs = [res.results[c]["out"] for c in range(cfg["NCORES"])]
    return np.concatenate(outs, axis=0)



# revision 6
# speedup vs baseline: 1.8606x; 1.8606x over previous
"""GAT (2-layer, PyG-style) on 8 Trainium2 NeuronCores via Bass/Tile. v2.

Differences vs the f32 baseline (same dst-sharded windowed-edge structure):
  - Everything bf16 on the PE and in the gather tables (4x matmul rate,
    2x DMA bytes). PSUM accumulation stays f32.
  - Rotated feature blocks: host builds per-head orthonormal-ish blocks
    B_h with column 0 = a_src1[h], table rows hold h@B (256 bf16 = 512B)
    so alpha_src is just column h*32 of the gathered row -- no separate
    aS gather and rows are exactly 512B. Post-scatter unmix by B^-1
    (2 matmuls/window). Same trick for layer 2 (M2, col 0 = a_src2).
  - alpha_dst per edge via a transposed one-hot T (dst-row-major) built
    on DVE from a DMA-broadcast d128T row, then per-chunk matmuls
    T_k @ aD_win -- removes the 256B-per-edge aD gather entirely
    (one third of baseline gather indices and SWDGE descgen time).
  - Layer-2 rows [rot-h2 (64) | pad] bf16 = 256B with a_src2 folded in;
    aD2 via the same T trick.
Gathers per window drop from 3 to 2 (lo/hi src half of the int16 index
space), gather bytes per edge from 1536B+ to 512B (L1) + 256B (L2).
"""
import sys

for _p in ("/opt/trn_rl_repo", "/opt/pypackages"):
    if _p not in sys.path:
        sys.path.insert(0, _p)

import numpy as np
from concourse import bacc, bass, mybir, tile
from concourse.masks import make_identity

P = 128
F32 = mybir.dt.float32
BF16 = mybir.dt.bfloat16
I16 = mybir.dt.int16
HALF = 32768

# ---- problem constants (nn_GAT_60000693125135) ----
N = 50000
IN_DIM = 256
H1 = 8
HID = 32
HC1 = H1 * HID  # 256
OUT = 64
NCORES = 8
NEG_SLOPE = 0.2

SP = False       # dma_gather single_packet
PMUL4D = True    # one 4D DVE op for the per-head p multiply


def _cdiv(a, b):
    return -(-a // b)


def _wrap16(vals, nidx):
    a = np.asarray(vals, np.int16).reshape(nidx // 16, 16).T
    return np.tile(a, (8, 1))


def _bf16(a):
    import ml_dtypes
    return np.asarray(a, np.float32).astype(ml_dtypes.bfloat16)


# ----------------------------------------------------------------------------
# Host-side preprocessing.
# ----------------------------------------------------------------------------
def prep_edges(edge_index, n, ncores):
    """Shard + window + src-half-split the edge list (self loops added).

    Per-core arrays: srclo16/srchi16 (wrap-16 gather idx), d128 [P, CT]
    (dst row in window, -1 on pads), d128T [nw, cmax*128] (transposed
    layout for the T build, -1 on pads)."""
    e0 = edge_index[0].astype(np.int64)
    e1 = edge_index[1].astype(np.int64)
    loops = np.arange(n, dtype=np.int64)
    src = np.concatenate([e0, loops])
    dst = np.concatenate([e1, loops])

    nchunk = n // ncores
    nw = _cdiv(nchunk, P)
    core = dst // nchunk
    dloc = dst - core * nchunk
    w = dloc // P
    hi = (src >= HALF).astype(np.int64)
    gid = (core * nw + w) * 2 + hi
    ngroups = ncores * nw * 2
    cnt = np.bincount(gid, minlength=ngroups).reshape(ncores, nw, 2)
    CWlo = _cdiv(cnt[:, :, 0].max(axis=0), P)
    CWhi = _cdiv(cnt[:, :, 1].max(axis=0), P)
    CW = CWlo + CWhi
    assert CW.min() >= 1
    CTlo, CThi, CT = int(CWlo.sum()), int(CWhi.sum()), int(CW.sum())
    cmax = int(CW.max())

    order = np.argsort(gid, kind="stable")
    gid_s = gid[order]
    starts = np.concatenate([[0], np.cumsum(np.bincount(gid_s, minlength=ngroups))])
    pos = np.arange(order.size) - starts[gid_s]
    c_s = core[order]
    w_s = w[order]
    hi_s = hi[order]
    slot = pos + hi_s * (CWlo[w_s] * P)

    srcv = np.zeros((ncores, nw, cmax * P), np.int64)
    d128v = np.full((ncores, nw, cmax * P), -1.0, np.float32)
    srcv[c_s, w_s, slot] = src[order] - hi_s * HALF
    d128v[c_s, w_s, slot] = (dloc[order] % P).astype(np.float32)

    srclo16 = np.zeros((ncores, P, 8 * CTlo), np.int16)
    srchi16 = np.zeros((ncores, P, 8 * CThi), np.int16)
    d128A = np.full((ncores, P, CT), -1.0, np.float32)
    d128T = np.full((ncores, nw, cmax * P), -1.0, np.float32)
    olo = ohi = oall = 0
    for wi in range(nw):
        nlo, nhi, nall = int(CWlo[wi]) * P, int(CWhi[wi]) * P, int(CW[wi]) * P
        for c in range(ncores):
            if nlo:
                srclo16[c, :, 8 * olo:8 * (olo + nlo // P)] = _wrap16(srcv[c, wi, :nlo], nlo)
            if nhi:
                srchi16[c, :, 8 * ohi:8 * (ohi + nhi // P)] = _wrap16(srcv[c, wi, nlo:nall], nhi)
            d128A[c, :, oall:oall + nall // P] = d128v[c, wi, :nall].reshape(nall // P, P).T
            d128T[c, wi, :nall] = d128v[c, wi, :nall]
        olo += nlo // P
        ohi += nhi // P
        oall += nall // P
    return dict(srclo16=srclo16, srchi16=srchi16, d128=d128A, d128T=d128T,
                CWlo=[int(v) for v in CWlo], CWhi=[int(v) for v in CWhi],
                cmax=cmax)


def _rot(a):
    """Invertible [d, d] block with column 0 == a, rest orthonormal."""
    a = np.asarray(a, np.float64)
    d = a.size
    q, _ = np.linalg.qr(np.column_stack([a, np.eye(d)[:, 1:]]))
    B = q.copy()
    B[:, 0] = a
    s = float(q[:, 0] @ a)
    Binv = q.T.copy()
    Binv[0, :] /= s
    assert np.abs(B @ Binv - np.eye(d)).max() < 1e-9
    return B.astype(np.float64), Binv.astype(np.float64)


# ----------------------------------------------------------------------------
# Kernel builder (SPMD program, same for all cores).
# ----------------------------------------------------------------------------
def build_nc(cfg):
    n = cfg["N"]; in_dim = cfg["IN"]; hc1 = cfg["HC1"]; h1 = cfg["H1"]
    out_dim = cfg["OUT"]; ncores = cfg["NCORES"]; neg = cfg["NEG"]
    CWlo, CWhi = cfg["CWlo"], cfg["CWhi"]
    CW = [a + b for a, b in zip(CWlo, CWhi)]
    cmax = cfg["cmax"]
    b1nz, b2nz = cfg["B1NZ"], cfg["B2NZ"]

    nchunk = n // ncores
    nw = _cdiv(nchunk, P)
    assert len(CW) == nw
    CTlo, CThi, CT = sum(CWlo), sum(CWhi), sum(CW)
    ntiles = _cdiv(n, P)
    npad = ntiles * P
    nlpad = nw * P
    kt1 = _cdiv(in_dim, P)   # 2
    ckt = _cdiv(hc1, P)      # 2
    NB = 8
    W2C = out_dim + 1        # 65: [rot-h2 | aD2]

    nc = bacc.Bacc(None, target_bir_lowering=False, debug=False,
                   num_devices=ncores)

    # ---- I/O ----
    xT_in = nc.dram_tensor("xT", [in_dim, npad], BF16, kind="ExternalInput")
    w1e_in = nc.dram_tensor("W1e", [in_dim, hc1 + h1], BF16, kind="ExternalInput")
    binv_in = nc.dram_tensor("Binv", [hc1, hc1], BF16, kind="ExternalInput")
    w2e_in = nc.dram_tensor("W2e", [hc1, W2C], BF16, kind="ExternalInput")
    m2i_in = nc.dram_tensor("M2i", [out_dim, out_dim], BF16, kind="ExternalInput")
    b1r_in = nc.dram_tensor("b1r", [P, hc1], BF16, kind="ExternalInput")
    b2r_in = nc.dram_tensor("b2r", [P, out_dim], F32, kind="ExternalInput")
    iota_in = nc.dram_tensor("iota", [P, P], BF16, kind="ExternalInput")
    iotac_in = nc.dram_tensor("iotac", [P, 1], F32, kind="ExternalInput")
    ones_in = nc.dram_tensor("ones1", [1, P], BF16, kind="ExternalInput")
    d128_in = nc.dram_tensor("d128", [P, CT], BF16, kind="ExternalInput")
    d128T_in = nc.dram_tensor("d128T", [nw, cmax * P], BF16, kind="ExternalInput")
    slo_in = nc.dram_tensor("srclo16", [P, 8 * CTlo], I16, kind="ExternalInput")
    shi_in = nc.dram_tensor("srchi16", [P, max(8 * CThi, 16)], I16, kind="ExternalInput")
    out_ext = nc.dram_tensor("out", [nchunk, out_dim], F32, kind="ExternalOutput")

    with tile.TileContext(nc) as tc:
        with (
            tc.tile_pool(name="dram", bufs=1, space="DRAM") as dram,
            tc.tile_pool(name="const", bufs=1) as cpool,
            tc.tile_pool(name="xst", bufs=2) as xpool,
            tc.tile_pool(name="hst", bufs=2) as hpool,
            tc.tile_pool(name="gbuf", bufs=6) as gpool,
            tc.tile_pool(name="g2buf", bufs=6) as g2pool,
            tc.tile_pool(name="sbuf", bufs=2) as spool,
            tc.tile_pool(name="tbuf", bufs=2) as tpool,
            tc.tile_pool(name="pebuf", bufs=2) as pepool,
            tc.tile_pool(name="o1", bufs=2) as o1pool,
            tc.tile_pool(name="small", bufs=3) as smpool,
            tc.tile_pool(name="psA", bufs=2, space="PSUM") as psA,
            tc.tile_pool(name="psB", bufs=1, space="PSUM") as psB,
            tc.tile_pool(name="psC", bufs=1, space="PSUM") as psC,
            tc.tile_pool(name="psD", bufs=1, space="PSUM") as psD,
            tc.tile_pool(name="psT", bufs=1, space="PSUM") as psT,
            tc.tile_pool(name="psQ", bufs=1, space="PSUM") as psQ,
        ):
            # ---- DRAM scratch ----
            h_tab = dram.tile([npad, hc1], BF16)
            aDfull = dram.tile([npad, h1], BF16)
            h2_mine = dram.tile([nchunk, 2 * out_dim], BF16)
            aD2_loc = dram.tile([nlpad, 1], BF16)
            h2_tab = dram.tile([n, 2 * out_dim], BF16, addr_space="Shared")

            # ---- resident constants ----
            iota_t = cpool.tile([P, P], BF16)
            nc.sync.dma_start(out=iota_t[:], in_=iota_in[:])
            iotac_t = cpool.tile([P, 1], F32)
            nc.sync.dma_start(out=iotac_t[:], in_=iotac_in[:])
            ones1 = cpool.tile([1, P], BF16)
            nc.sync.dma_start(out=ones1[:], in_=ones_in[:])
            d128_t = cpool.tile([P, CT], BF16)
            nc.sync.dma_start(out=d128_t[:], in_=d128_in[:])
            ident = cpool.tile([P, P], BF16)
            make_identity(nc, ident[:])
            b1r = cpool.tile([P, hc1], BF16)
            nc.sync.dma_start(out=b1r[:], in_=b1r_in[:])
            b2r = cpool.tile([P, out_dim], F32)
            nc.sync.dma_start(out=b2r[:], in_=b2r_in[:])
            slo = cpool.tile([P, 8 * CTlo], I16)
            nc.sync.dma_start(out=slo[:], in_=slo_in[:])
            shi = cpool.tile([P, max(8 * CThi, 16)], I16)
            nc.sync.dma_start(out=shi[:], in_=shi_in[:])
            w1e = cpool.tile([P, kt1, hc1 + h1], BF16)
            for kt in range(kt1):
                nc.sync.dma_start(out=w1e[:, kt, :], in_=w1e_in[kt * P:(kt + 1) * P, :])
            binv = cpool.tile([P, ckt, hc1], BF16)
            for c in range(ckt):
                nc.sync.dma_start(out=binv[:, c, :], in_=binv_in[c * P:(c + 1) * P, :])
            w2e = cpool.tile([P, ckt, W2C], BF16)
            for c in range(ckt):
                nc.sync.dma_start(out=w2e[:, c, :], in_=w2e_in[c * P:(c + 1) * P, :])
            m2i = cpool.tile([out_dim, out_dim], BF16)
            nc.sync.dma_start(out=m2i[:], in_=m2i_in[:])

            # ---- phase 1: h_ext = x @ w1e -> h_tab + aDfull (replicated) ----
            for g in range(_cdiv(ntiles, NB)):
                nt0 = g * NB
                nb = min(NB, ntiles - nt0)
                xst = xpool.tile([P, kt1, NB * P], BF16, tag="xst")
                for kt in range(kt1):
                    nc.sync.dma_start(out=xst[:, kt, 0:nb * P],
                                      in_=xT_in[kt * P:(kt + 1) * P, nt0 * P:(nt0 + nb) * P])
                hstg = hpool.tile([P, NB, hc1], BF16, tag="hst")
                astg = hpool.tile([P, NB, h1], BF16, tag="ast")
                for j in range(nb):
                    ps = psA.tile([P, hc1 + h1], F32, tag="ops")
                    for kt in range(kt1):
                        nc.tensor.matmul(out=ps[:], lhsT=xst[:, kt, j * P:(j + 1) * P],
                                         rhs=w1e[:, kt, :], start=(kt == 0), stop=(kt == kt1 - 1))
                    nc.scalar.copy(out=hstg[:, j, :], in_=ps[:, 0:hc1])
                    nc.scalar.copy(out=astg[:, j, :], in_=ps[:, hc1:])
                hv = h_tab[nt0 * P:(nt0 + nb) * P, :].rearrange("(j p) c -> p j c", p=P)
                nc.sync.dma_start(out=hv, in_=hstg[:, 0:nb, :])
                av = aDfull[nt0 * P:(nt0 + nb) * P, :].rearrange("(j p) c -> p j c", p=P)
                nc.sync.dma_start(out=av, in_=astg[:, 0:nb, :])

            pid_rows = nc.sync.snap(nc.sync.partition_id() * nchunk)
            stop = cfg.get("STOP", "")

            def bounce_out(src_dram, width):
                for w in range(nw):
                    rows = min(P, nchunk - w * P)
                    dbg = smpool.tile([P, out_dim], F32, tag="dbg")
                    nc.vector.tensor_scalar(
                        out=dbg[:rows, :],
                        in0=src_dram[w * P:w * P + rows, 0:width],
                        scalar1=1.0, scalar2=None, op0=mybir.AluOpType.mult)
                    nc.sync.dma_start(out=out_ext[w * P:w * P + rows, :],
                                      in_=dbg[:rows, :])

            if stop == "phase1":
                # bounce own-chunk h_tab rows (rotated) for host check
                for w in range(nw):
                    rows = min(P, nchunk - w * P)
                    dbg = smpool.tile([P, out_dim], F32, tag="dbg")
                    src = h_tab[bass.ds(pid_rows + w * P, rows), 0:out_dim]
                    sb = smpool.tile([P, out_dim], BF16, tag="dbgb")
                    nc.sync.dma_start(out=sb[:rows, :], in_=src)
                    nc.vector.tensor_scalar(out=dbg[:rows, :], in0=sb[:rows, :],
                                            scalar1=1.0, scalar2=None,
                                            op0=mybir.AluOpType.mult)
                    nc.sync.dma_start(out=out_ext[w * P:w * P + rows, :],
                                      in_=dbg[:rows, :])
                return nc

            # ---- phase 2: layer-1 edge aggregation per dst window ----
            olo = oall = 0
            for w in range(nw):
                Clo, Chi, C = CWlo[w], CWhi[w], CW[w]
                rows = min(P, nchunk - w * P)
                G = gpool.tile([P, cmax, hc1], BF16, tag="G")
                if Clo:
                    nc.gpsimd.dma_gather(
                        out_ap=G[:, 0:Clo, :], in_ap=h_tab[:],
                        idxs_ap=slo[:, 8 * olo:8 * (olo + Clo)],
                        num_idxs=Clo * P, num_idxs_reg=Clo * P, elem_size=hc1,
                        single_packet=SP)
                if Chi:
                    nc.gpsimd.dma_gather(
                        out_ap=G[:, Clo:C, :], in_ap=h_tab[HALF:, :],
                        idxs_ap=shi[:, 8 * (oall - olo):8 * (oall - olo + Chi)],
                        num_idxs=Chi * P, num_idxs_reg=Chi * P, elem_size=hc1,
                        single_packet=SP)
                aDw = smpool.tile([P, h1], BF16, tag="aDw")
                nc.sync.dma_start(out=aDw[:rows, :],
                                  in_=aDfull[bass.ds(pid_rows + w * P, rows), :])
                S = spool.tile([P, cmax, P], BF16, tag="S")
                nc.vector.tensor_tensor(
                    out=S[:, 0:C, :],
                    in0=d128_t[:, oall:oall + C].unsqueeze(-1).to_broadcast((P, C, P)),
                    in1=iota_t[:].unsqueeze(1).to_broadcast((P, C, P)),
                    op=mybir.AluOpType.is_equal)
                dT = tpool.tile([1, cmax * P], BF16, tag="dT")
                nc.sync.dma_start(out=dT[:, 0:C * P], in_=d128T_in[w:w + 1, 0:C * P])
                T = tpool.tile([P, cmax, P], BF16, tag="T")
                for g0 in range(0, C, 8):
                    gn = min(8, C - g0)
                    Qps = psQ.tile([P, 8 * P], F32, tag="q")
                    for j in range(gn):
                        nc.tensor.matmul(
                            out=Qps[:, j * P:(j + 1) * P], lhsT=ones1[:],
                            rhs=dT[0:1, (g0 + j) * P:(g0 + j + 1) * P],
                            start=True, stop=True)
                    nc.vector.tensor_tensor(
                        out=T[:, g0:g0 + gn, :],
                        in0=Qps[:, 0:gn * P].rearrange("r (k q) -> r k q", k=gn),
                        in1=iotac_t[:, 0:1].unsqueeze(1).to_broadcast((P, gn, P)),
                        op=mybir.AluOpType.is_equal)
                aDps = psD.tile([P, cmax, h1], F32, tag="aD")
                for k in range(C):
                    nc.tensor.matmul(out=aDps[:, k, :], lhsT=T[:, k, :], rhs=aDw[:],
                                     start=True, stop=True)
                aDsb = smpool.tile([P, cmax, h1], BF16, tag="aDsb")
                nc.scalar.copy(out=aDsb[:, 0:C, :], in_=aDps[:, 0:C, :])
                pe = pepool.tile([P, cmax, h1], BF16, tag="pe")
                G4 = G[:, 0:C, :].rearrange("p c (h j) -> p c h j", h=h1)
                nc.vector.tensor_tensor(
                    out=pe[:, 0:C, :], in0=G4[:, :, :, 0].squeeze(),
                    in1=aDsb[:, 0:C, :], op=mybir.AluOpType.add)
                nc.vector.scalar_tensor_tensor(
                    out=pe[:, 0:C, :], in0=pe[:, 0:C, :], scalar=neg,
                    in1=pe[:, 0:C, :], op0=mybir.AluOpType.mult,
                    op1=mybir.AluOpType.max)
                nc.scalar.activation(out=pe[:, 0:C, :], in_=pe[:, 0:C, :],
                                     func=mybir.ActivationFunctionType.Exp)
                if PMUL4D:
                    nc.vector.tensor_tensor(
                        out=G4, in0=G4,
                        in1=pe[:, 0:C, :].unsqueeze(-1).to_broadcast((P, C, h1, HID)),
                        op=mybir.AluOpType.mult)
                else:
                    for h in range(h1):
                        nc.vector.tensor_tensor(
                            out=G[:, 0:C, h * HID:(h + 1) * HID],
                            in0=G[:, 0:C, h * HID:(h + 1) * HID],
                            in1=pe[:, 0:C, h:h + 1].to_broadcast((P, C, HID)),
                            op=mybir.AluOpType.mult)
                ops = psA.tile([P, hc1 + h1], F32, tag="ops")
                for k in range(C):
                    nc.tensor.matmul(out=ops[:, 0:hc1], lhsT=S[:, k, :], rhs=G[:, k, :],
                                     start=(k == 0), stop=(k == C - 1))
                    nc.tensor.matmul(out=ops[:, hc1:], lhsT=S[:, k, :], rhs=pe[:, k, :],
                                     start=(k == 0), stop=(k == C - 1))
                rec = smpool.tile([P, h1], F32, tag="rec")
                nc.vector.reciprocal(out=rec[:], in_=ops[:, hc1:])
                opssb = o1pool.tile([P, hc1], BF16, tag="opssb")
                nc.scalar.copy(out=opssb[:], in_=ops[:, 0:hc1])
                ats = []
                for c in range(ckt):
                    tp = psT.tile([P, P], BF16, tag="tp")
                    nc.tensor.transpose(tp[:], opssb[:, c * P:(c + 1) * P], ident[:])
                    at = o1pool.tile([P, P], BF16, tag="at")
                    nc.scalar.copy(out=at[:], in_=tp[:])
                    ats.append(at)
                h1u = psB.tile([P, hc1], F32, tag="h1u")
                for c in range(ckt):
                    nc.tensor.matmul(out=h1u[:], lhsT=ats[c][:], rhs=binv[:, c, :],
                                     start=(c == 0), stop=(c == ckt - 1))
                h1w = o1pool.tile([P, hc1], BF16, tag="h1w")
                nc.vector.tensor_tensor(
                    out=h1w[:].rearrange("p (h j) -> p h j", h=h1),
                    in0=h1u[:].rearrange("p (h j) -> p h j", h=h1),
                    in1=rec[:].unsqueeze(-1).to_broadcast((P, h1, HID)),
                    op=mybir.AluOpType.mult)
                if b1nz:
                    nc.vector.tensor_tensor(out=h1w[:], in0=h1w[:], in1=b1r[:],
                                            op=mybir.AluOpType.add)
                nc.vector.tensor_scalar(out=h1w[:], in0=h1w[:], scalar1=0.0,
                                        scalar2=None, op0=mybir.AluOpType.max)
                ats2 = []
                for c in range(ckt):
                    tp = psT.tile([P, P], BF16, tag="tp")
                    nc.tensor.transpose(tp[:], h1w[:, c * P:(c + 1) * P], ident[:])
                    at = o1pool.tile([P, P], BF16, tag="at")
                    nc.scalar.copy(out=at[:], in_=tp[:])
                    ats2.append(at)
                h2e = psC.tile([P, W2C], F32, tag="h2e")
                for c in range(ckt):
                    nc.tensor.matmul(out=h2e[:], lhsT=ats2[c][:], rhs=w2e[:, c, :],
                                     start=(c == 0), stop=(c == ckt - 1))
                h2sb = o1pool.tile([P, 2 * out_dim], BF16, tag="h2sb")
                nc.scalar.copy(out=h2sb[:, 0:out_dim], in_=h2e[:, 0:out_dim])
                nc.sync.dma_start(out=h2_mine[w * P:w * P + rows, :], in_=h2sb[:rows, :])
                a2sb = smpool.tile([P, 1], BF16, tag="a2sb")
                nc.scalar.copy(out=a2sb[:], in_=h2e[:, out_dim:out_dim + 1])
                nc.sync.dma_start(out=aD2_loc[w * P:w * P + rows, :], in_=a2sb[:rows, :])
                olo += Clo
                oall += C

            if stop == "phase2":
                bounce_out(h2_mine, out_dim)
                return nc

            # ---- all-gather h2 ----
            nc.gpsimd.collective_compute(
                "AllGather", mybir.AluOpType.bypass,
                replica_groups=[list(range(ncores))],
                ins=[h2_mine[:].opt()], outs=[h2_tab[:].opt()])

            if stop == "cc":
                bounce_out(h2_tab, out_dim)
                return nc

            # ---- phase 3: layer-2 edge aggregation + log_softmax ----
            t_all = cpool.tile([P, nw, out_dim], F32)
            s_all = cpool.tile([P, nw], F32)
            olo = oall = 0
            for w in range(nw):
                Clo, Chi, C = CWlo[w], CWhi[w], CW[w]
                rows = min(P, nchunk - w * P)
                G2 = g2pool.tile([P, cmax, 2 * out_dim], BF16, tag="G2")
                if Clo:
                    nc.gpsimd.dma_gather(
                        out_ap=G2[:, 0:Clo, :], in_ap=h2_tab[:],
                        idxs_ap=slo[:, 8 * olo:8 * (olo + Clo)],
                        num_idxs=Clo * P, num_idxs_reg=Clo * P,
                        elem_size=2 * out_dim, single_packet=SP)
                if Chi:
                    nc.gpsimd.dma_gather(
                        out_ap=G2[:, Clo:C, :], in_ap=h2_tab[HALF:, :],
                        idxs_ap=shi[:, 8 * (oall - olo):8 * (oall - olo + Chi)],
                        num_idxs=Chi * P, num_idxs_reg=Chi * P,
                        elem_size=2 * out_dim, single_packet=SP)
                aD2w = smpool.tile([P, 1], BF16, tag="aD2w")
                nc.sync.dma_start(out=aD2w[:rows, :],
                                  in_=aD2_loc[w * P:w * P + rows, :])
                S = spool.tile([P, cmax, P], BF16, tag="S")
                nc.vector.tensor_tensor(
                    out=S[:, 0:C, :],
                    in0=d128_t[:, oall:oall + C].unsqueeze(-1).to_broadcast((P, C, P)),
                    in1=iota_t[:].unsqueeze(1).to_broadcast((P, C, P)),
                    op=mybir.AluOpType.is_equal)
                dT = tpool.tile([1, cmax * P], BF16, tag="dT")
                nc.sync.dma_start(out=dT[:, 0:C * P], in_=d128T_in[w:w + 1, 0:C * P])
                T = tpool.tile([P, cmax, P], BF16, tag="T")
                for g0 in range(0, C, 8):
                    gn = min(8, C - g0)
                    Qps = psQ.tile([P, 8 * P], F32, tag="q")
                    for j in range(gn):
                        nc.tensor.matmul(
                            out=Qps[:, j * P:(j + 1) * P], lhsT=ones1[:],
                            rhs=dT[0:1, (g0 + j) * P:(g0 + j + 1) * P],
                            start=True, stop=True)
                    nc.vector.tensor_tensor(
                        out=T[:, g0:g0 + gn, :],
                        in0=Qps[:, 0:gn * P].rearrange("r (k q) -> r k q", k=gn),
                        in1=iotac_t[:, 0:1].unsqueeze(1).to_broadcast((P, gn, P)),
                        op=mybir.AluOpType.is_equal)
                aD2ps = psD.tile([P, cmax, 1], F32, tag="aD")
                for k in range(C):
                    nc.tensor.matmul(out=aD2ps[:, k, :], lhsT=T[:, k, :], rhs=aD2w[:],
                                     start=True, stop=True)
                aD2sb = smpool.tile([P, cmax], BF16, tag="aD2sb")
                nc.scalar.copy(out=aD2sb[:, 0:C], in_=aD2ps[:, 0:C, 0].squeeze())
                pe2 = pepool.tile([P, cmax], BF16, tag="pe2")
                nc.vector.tensor_tensor(
                    out=pe2[:, 0:C], in0=G2[:, 0:C, 0].squeeze(),
                    in1=aD2sb[:, 0:C], op=mybir.AluOpType.add)
                nc.vector.scalar_tensor_tensor(
                    out=pe2[:, 0:C], in0=pe2[:, 0:C], scalar=neg,
                    in1=pe2[:, 0:C], op0=mybir.AluOpType.mult,
                    op1=mybir.AluOpType.max)
                nc.scalar.activation(out=pe2[:, 0:C], in_=pe2[:, 0:C],
                                     func=mybir.ActivationFunctionType.Exp)
                G2b = pepool.tile([P, cmax, out_dim], BF16, tag="G2b")
                nc.vector.tensor_tensor(
                    out=G2b[:, 0:C, :], in0=G2[:, 0:C, 0:out_dim],
                    in1=pe2[:, 0:C].unsqueeze(-1).to_broadcast((P, C, out_dim)),
                    op=mybir.AluOpType.mult)
                ops2 = psA.tile([P, out_dim + 1], F32, tag="ops")
                for k in range(C):
                    nc.tensor.matmul(out=ops2[:, 0:out_dim], lhsT=S[:, k, :],
                                     rhs=G2b[:, k, :], start=(k == 0), stop=(k == C - 1))
                    nc.tensor.matmul(out=ops2[:, out_dim:], lhsT=S[:, k, :],
                                     rhs=pe2[:, k:k + 1], start=(k == 0), stop=(k == C - 1))
                rec2 = smpool.tile([P, 1], F32, tag="rec2")
                nc.vector.reciprocal(out=rec2[:], in_=ops2[:, out_dim:])
                o2sb = o1pool.tile([P, out_dim], BF16, tag="o2sb")
                nc.scalar.copy(out=o2sb[:], in_=ops2[:, 0:out_dim])
                tp = psT.tile([P, P], BF16, tag="tp")
                nc.tensor.transpose(tp[0:out_dim, :], o2sb[:], ident[:])
                at5 = o1pool.tile([out_dim, P], BF16, tag="at5")
                nc.scalar.copy(out=at5[:], in_=tp[0:out_dim, :])
                z = psB.tile([P, out_dim], F32, tag="h1u")
                nc.tensor.matmul(out=z[:], lhsT=at5[:], rhs=m2i[:],
                                 start=True, stop=True)
                zf = smpool.tile([P, out_dim], F32, tag="zf")
                nc.vector.tensor_tensor(out=zf[:], in0=z[:],
                                        in1=rec2[:].to_broadcast((P, out_dim)),
                                        op=mybir.AluOpType.mult)
                if b2nz:
                    nc.vector.tensor_tensor(out=zf[:], in0=zf[:], in1=b2r[:],
                                            op=mybir.AluOpType.add)
                negmax = smpool.tile([P, 1], F32, tag="negmax")
                nc.vector.tensor_reduce(out=negmax[:], in_=zf[:],
                                        axis=mybir.AxisListType.X,
                                        op=mybir.AluOpType.max, negate=True)
                nc.vector.tensor_scalar(out=t_all[:, w, :], in0=zf[:],
                                        scalar1=negmax[:], scalar2=None,
                                        op0=mybir.AluOpType.add)
                esc = smpool.tile([P, out_dim], F32, tag="esc")
                nc.scalar.activation(out=esc[:], in_=t_all[:, w, :],
                                     func=mybir.ActivationFunctionType.Exp,
                                     accum_out=s_all[:, w:w + 1])
                olo += Clo
                oall += C
            lns = cpool.tile([P, nw], F32)
            nc.scalar.activation(out=lns[:], in_=s_all[:],
                                 func=mybir.ActivationFunctionType.Ln)
            for w in range(nw):
                rows = min(P, nchunk - w * P)
                res = smpool.tile([P, out_dim], F32, tag="esc")
                nc.vector.tensor_scalar(out=res[:], in0=t_all[:, w, :],
                                        scalar1=lns[:, w:w + 1], scalar2=None,
                                        op0=mybir.AluOpType.subtract)
                nc.sync.dma_start(out=out_ext[w * P:w * P + rows, :], in_=res[:rows, :])

    return nc


# ----------------------------------------------------------------------------
# Host-side input packing.
# ----------------------------------------------------------------------------
def make_in_maps(inputs, cfg):
    n = cfg["N"]; in_dim = cfg["IN"]; hc1 = cfg["HC1"]; h1 = cfg["H1"]
    hid = cfg["HID"]; out_dim = cfg["OUT"]; ncores = cfg["NCORES"]

    x = np.asarray(inputs["x"], np.float32)
    ei = np.asarray(inputs["edge_index"])
    W1 = np.asarray(inputs["W1"], np.float64)
    a_src1 = np.asarray(inputs["a_src1"], np.float64)
    a_dst1 = np.asarray(inputs["a_dst1"], np.float64)
    b1 = np.asarray(inputs["b1"], np.float32)
    W2 = np.asarray(inputs["W2"], np.float64)
    a_src2 = np.asarray(inputs["a_src2"], np.float64)
    a_dst2 = np.asarray(inputs["a_dst2"], np.float64)
    b2 = np.asarray(inputs["b2"], np.float32)

    cfg["B1NZ"] = bool(np.any(b1))
    cfg["B2NZ"] = bool(np.any(b2))

    ntiles = _cdiv(n, P)
    npad = ntiles * P
    xT = np.zeros((in_dim, npad), np.float32)
    xT[:, :n] = x.T

    # rotation blocks: B_h col 0 = a_src1[h]
    W1e = np.zeros((in_dim, hc1 + h1), np.float64)
    Binv = np.zeros((hc1, hc1), np.float64)
    for h in range(h1):
        B, Bi = _rot(a_src1[h])
        W1e[:, h * hid:(h + 1) * hid] = W1[:, h * hid:(h + 1) * hid] @ B
        W1e[:, hc1 + h] = W1[:, h * hid:(h + 1) * hid] @ a_dst1[h]
        Binv[h * hid:(h + 1) * hid, h * hid:(h + 1) * hid] = Bi
    M2, M2i = _rot(a_src2[0])
    W2e = np.zeros((hc1, out_dim + 1), np.float64)
    W2e[:, 0:out_dim] = W2 @ M2
    W2e[:, out_dim] = W2 @ a_dst2[0]

    pe = prep_edges(ei, n, ncores)
    cfg["CWlo"], cfg["CWhi"], cfg["cmax"] = pe["CWlo"], pe["CWhi"], pe["cmax"]

    iota = np.tile(np.arange(P, dtype=np.float32)[None, :], (P, 1))
    common = {
        "xT": _bf16(xT),
        "W1e": _bf16(W1e), "Binv": _bf16(Binv),
        "W2e": _bf16(W2e), "M2i": _bf16(M2i),
        "b1r": _bf16(np.tile(b1[None, :], (P, 1))),
        "b2r": np.tile(b2[None, :], (P, 1)).astype(np.float32),
        "iota": _bf16(iota),
        "iotac": np.arange(P, dtype=np.float32)[:, None],
        "ones1": _bf16(np.ones((1, P), np.float32)),
    }
    in_maps = []
    for c in range(ncores):
        m = dict(common)
        m["srclo16"] = np.ascontiguousarray(pe["srclo16"][c])
        shi = pe["srchi16"][c]
        if shi.shape[1] == 0:
            shi = np.zeros((P, 16), np.int16)
        m["srchi16"] = np.ascontiguousarray(shi)
        m["d128"] = _bf16(pe["d128"][c])
        m["d128T"] = _bf16(pe["d128T"][c])
        in_maps.append(m)
    return in_maps


DEFAULT_CFG = dict(N=N, IN=IN_DIM, HC1=HC1, H1=H1, HID=HID, OUT=OUT,
                   NCORES=NCORES, NEG=NEG_SLOPE)

TRACE = False
LAST_RESULTS = None


def kernel(**inputs) -> np.ndarray:
    global LAST_RESULTS
    from concourse.bass_utils import run_bass_kernel_spmd

    cfg = dict(DEFAULT_CFG)
    in_maps = make_in_maps(inputs, cfg)
    nc = build_nc(cfg)
    if not nc.is_finalized():
        nc.finalize()
    res = run_bass_kernel_spmd(nc, in_maps, core_ids=list(range(cfg["NCORES"])),
                               trace=TRACE)
    LAST_RESULTS = res
    outs = [res.results[c]["out"] for c in range(cfg["NCORES"])]
    return np.concatenate(outs, axis=0).astype(np.float32)
